# revision 6
# baseline (speedup 1.0000x reference)
"""CrossAttention kernel for TRN2, 8 NeuronCores, batch data-parallel.

Full inputs in, full output out. Sharding: batch 16 -> 2 batches/core, no
collectives. Per core everything runs in the "transposed world": activations
are stored [feature, token] so every matmul's contraction dim sits on
partitions.

  per token-chunk (512 tokens), per inner-tile it (= head pair):
    QT_tile = sum_kk wqT[kk,it]-stationary @ xT[kk]        psum [128, 512]
    per head h in pair:
      S^T   = KT_head (stationary) @ QT_tile[head rows]    psum [77, 512]
      P^T   = exp(S^T * scale)   (no max-shift; logits are O(5))
      [O^T; sumexp] = V_aug_head (stationary, ones col) @ P^T   psum [65, 512]
      A^T rows <- O^T;  rcp = 1/sumexp -> DRAM
    A^T[it] *= broadcast(rcp)     (DMA partition-broadcast from DRAM)
  out = A^T-chunks (stationary) @ woT (streaming) + bo     [tok, 1280]

All matmuls run in float32r (1 cycle/row at N>=256, rel-err ~2e-4).
"""

import numpy as np

HEADS = 20
DH = 64
QD = 1280
CD = 2048
INNER = 1280
B = 16
NQ = 4096
NK = 77
SCALE = DH ** -0.5
NCORES = 8
BPC = B // NCORES          # batches per core
TCH = 512                  # token chunk == matmul free dim
NCHUNK = NQ // TCH         # chunks per batch
P = 128
NKK_Q = QD // P            # 10 contraction blocks for Q proj
NKK_C = CD // P            # 16 contraction blocks for K/V proj
NIT = INNER // P           # 10 inner tiles (head pairs)
OBW = [512, 512, 256]      # output-feature blocks
OBO = [0, 512, 1024]

_CACHED = {}


def _build_nc():
    import contextlib

    import concourse.bacc as bacc
    import concourse.mybir as mybir
    import concourse.tile as tile

    f32 = mybir.dt.float32
    f32r = mybir.dt.float32r
    EXP = mybir.ActivationFunctionType.Exp

    nc = bacc.Bacc()
    xT = nc.declare_dram_parameter("xT", [BPC * QD, NQ], f32r, isOutput=False)
    ctxT = nc.declare_dram_parameter("ctxT", [BPC * CD, NK], f32r, isOutput=False)
    wqT = nc.declare_dram_parameter("wqT", [QD, INNER], f32r, isOutput=False)
    wkT = nc.declare_dram_parameter("wkT", [CD, INNER], f32r, isOutput=False)
    wvT = nc.declare_dram_parameter("wvT", [CD, INNER], f32r, isOutput=False)
    woT = nc.declare_dram_parameter("woT", [INNER, QD], f32r, isOutput=False)
    bo = nc.declare_dram_parameter("bo", [1, QD], f32, isOutput=False)
    out = nc.declare_dram_parameter("out", [BPC * NQ, QD], f32, isOutput=True)

    with tile.TileContext(nc) as tc, contextlib.ExitStack() as ctx:
        sb = ctx.enter_context(tc.tile_pool(name="sb", bufs=1))
        st = ctx.enter_context(tc.tile_pool(name="st", bufs=3))
        dr = ctx.enter_context(tc.tile_pool(name="dr", bufs=2, space="DRAM"))

        # resident: bias broadcast + full wqT
        bias_sb = sb.tile([P, QD], f32, name="bias")
        nc.sync.dma_start(bias_sb[:], bo[:].to_broadcast((P, QD)))
        wq_sb = [sb.tile([P, INNER], f32r, name=f"wq{k}") for k in range(NKK_Q)]
        for k in range(NKK_Q):
            nc.sync.dma_start(wq_sb[k][:], wqT[k * P : (k + 1) * P, :])

        for b in range(BPC):
            # ---- K^T [1280, 77] and V_aug [77, 65*HEADS] for batch b ----
            ctx_sb = [sb.tile([P, 80], f32r, name=f"ctx{k}") for k in range(NKK_C)]
            for k in range(NKK_C):
                nc.vector.memset(ctx_sb[k][:].bitcast(f32), 0)
                nc.sync.dma_start(
                    ctx_sb[k][:, :NK], ctxT[b * CD + k * P : b * CD + (k + 1) * P, :]
                )
            kt_sb = [sb.tile([P, 80], f32r, name=f"kt{i}") for i in range(NIT)]
            vaug = sb.tile([NK, 65 * HEADS], f32r, name="vaug")
            ones_sb = sb.tile([NK, 1], f32, name="ones_sb")
            nc.vector.memset(ones_sb[:], 1.0)
            for h in range(HEADS):
                nc.vector.tensor_copy(
                    vaug[:, h * 65 + DH : h * 65 + 65], ones_sb[:]
                )
            with tc.tile_pool(name=f"pskv{b}", bufs=1, space="PSUM") as pskv:
                for g in range(2):  # two it-groups of 5 to bound psum
                    kps = [
                        pskv.tile([P, 80], f32, name=f"kps{i}", bufs=1)
                        for i in range(5)
                    ]
                    for k in range(NKK_C):
                        wk_t = st.tile([P, INNER], f32r, name="wkv_st", bufs=2)
                        nc.sync.dma_start(wk_t[:], wkT[k * P : (k + 1) * P, :])
                        for i in range(5):
                            it = g * 5 + i
                            nc.tensor.matmul(
                                kps[i][:],
                                wk_t[:, it * P : (it + 1) * P],
                                ctx_sb[k][:],
                                start=(k == 0),
                                stop=(k == NKK_C - 1),
                            )
                    for i in range(5):
                        nc.vector.tensor_copy(kt_sb[g * 5 + i][:], kps[i][:])
                vps = [
                    pskv.tile([NK, OBW[j]], f32, name=f"vps{j}", bufs=1)
                    for j in range(3)
                ]
                for k in range(NKK_C):
                    wv_t = st.tile([P, INNER], f32r, name="wkv_st", bufs=2)
                    nc.sync.dma_start(wv_t[:], wvT[k * P : (k + 1) * P, :])
                    for j in range(3):
                        nc.tensor.matmul(
                            vps[j][:],
                            ctx_sb[k][:, :NK],
                            wv_t[:, OBO[j] : OBO[j] + OBW[j]],
                            start=(k == 0),
                            stop=(k == NKK_C - 1),
                        )
                for h in range(HEADS):
                    j = (h * DH) // 512
                    o = (h * DH) % 512
                    nc.vector.tensor_copy(
                        vaug[:, h * 65 : h * 65 + DH], vps[j][:, o : o + DH]
                    )
            # wo blocks for this batch (resident across its chunks)
            wo_sb = [sb.tile([P, QD], f32r, name=f"wo{k}") for k in range(NIT)]
            for k in range(NIT):
                nc.sync.dma_start(wo_sb[k][:], woT[k * P : (k + 1) * P, :])

            for ch in range(NCHUNK):
                coff = ch * TCH
                xt = [
                    st.tile([P, TCH], f32r, name=f"xt{k}", bufs=1)
                    for k in range(NKK_Q)
                ]
                for k in range(NKK_Q):
                    nc.sync.dma_start(
                        xt[k][:],
                        xT[b * QD + k * P : b * QD + (k + 1) * P, coff : coff + TCH],
                    )
                at = [sb.tile([P, TCH], f32r, name=f"at{i}") for i in range(NIT)]
                rcp_d = dr.tile([HEADS, TCH], f32, name="rcp_d")

                # ---- fused Q-projection + attention ----
                with tc.tile_pool(
                    name=f"psa{b}_{ch}", bufs=1, space="PSUM"
                ) as psa:
                    for it in range(NIT):
                        psq = psa.tile([P, TCH], f32, name="psq", bufs=2)
                        for kk in range(NKK_Q):
                            nc.tensor.matmul(
                                psq[:],
                                wq_sb[kk][:, it * P : (it + 1) * P],
                                xt[kk][:],
                                start=(kk == 0),
                                stop=(kk == NKK_Q - 1),
                            )
                        qt_t = st.tile([P, TCH], f32r, name="qt_t")
                        nc.vector.tensor_copy(qt_t[:], psq[:])
                        for hh in range(2):
                            h = 2 * it + hh
                            hb = hh * DH
                            ps_s = psa.tile([80, TCH], f32, name="ps_s", bufs=2)
                            nc.tensor.matmul(
                                ps_s[:],
                                kt_sb[it][hb : hb + DH, :],
                                qt_t[hb : hb + DH, :],
                                start=True,
                                stop=True,
                            )
                            expst = st.tile([80, TCH], f32r, name="expst")
                            nc.scalar.activation(
                                expst[:], ps_s[:], EXP, scale=SCALE
                            )
                            ps_o = psa.tile([65, TCH], f32, name="ps_o", bufs=2)
                            nc.tensor.matmul(
                                ps_o[:],
                                vaug[:, h * 65 : (h + 1) * 65],
                                expst[0:NK, :],
                                start=True,
                                stop=True,
                            )
                            nc.any.tensor_copy(
                                at[it][hb : hb + DH, :], ps_o[0:DH, :]
                            )
                            rcp_row = st.tile([65, TCH], f32, name="rcp_row", bufs=2)
                            nc.vector.reciprocal(rcp_row[64:65, :], ps_o[64:65, :])
                            nc.gpsimd.dma_start(
                                rcp_d[h : h + 1, :], rcp_row[64:65, :]
                            )
                        # normalize both heads of at[it]
                        rcpb = st.tile([P, TCH], f32, name="rcpb")
                        nc.sync.dma_start(
                            rcpb[0:DH, :],
                            rcp_d[2 * it : 2 * it + 1, :].to_broadcast((DH, TCH)),
                        )
                        nc.sync.dma_start(
                            rcpb[DH:P, :],
                            rcp_d[2 * it + 1 : 2 * it + 2, :].to_broadcast(
                                (DH, TCH)
                            ),
                        )
                        nc.vector.tensor_mul(at[it][:], at[it][:], rcpb[:])

                # ---- output projection for chunk ----
                with tc.tile_pool(
                    name=f"pso{b}_{ch}", bufs=1, space="PSUM"
                ) as pso:
                    for tt in range(TCH // P):
                        pos = [
                            pso.tile([P, OBW[j]], f32, name=f"po{j}", bufs=2)
                            for j in range(3)
                        ]
                        for kk in range(NIT):
                            for j in range(3):
                                nc.tensor.matmul(
                                    pos[j][:],
                                    at[kk][:, tt * P : (tt + 1) * P],
                                    wo_sb[kk][
                                        :, OBO[j] : OBO[j] + OBW[j]
                                    ],
                                    start=(kk == 0),
                                    stop=(kk == NIT - 1),
                                )
                        ost = st.tile([P, QD], f32, name="ost", bufs=2)
                        for j in range(3):
                            nc.any.tensor_add(
                                ost[:, OBO[j] : OBO[j] + OBW[j]],
                                pos[j][:],
                                bias_sb[:, OBO[j] : OBO[j] + OBW[j]],
                            )
                        nc.gpsimd.dma_start(
                            out[
                                b * NQ + coff + tt * P : b * NQ
                                + coff
                                + (tt + 1) * P,
                                :,
                            ],
                            ost[:],
                        )

    nc.finalize()
    return nc


def _prep_inputs(x, context, vq, gq, vk, gk, vv, gv, vo, go, bo):
    def wn(v, g):
        n = np.sqrt(np.sum(v * v, axis=1, keepdims=True))
        return v * (g[:, None] / n)

    wqT = np.ascontiguousarray(wn(vq, gq).T.astype(np.float32))   # [QD, INNER]
    wkT = np.ascontiguousarray(wn(vk, gk).T.astype(np.float32))   # [CD, INNER]
    wvT = np.ascontiguousarray(wn(vv, gv).T.astype(np.float32))   # [CD, INNER]
    woT = np.ascontiguousarray(wn(vo, go).T.astype(np.float32))   # [INNER, QD]
    bo2 = np.ascontiguousarray(bo.reshape(1, QD).astype(np.float32))

    in_maps = []
    for c in range(NCORES):
        xs = np.asarray(x[c * BPC : (c + 1) * BPC], dtype=np.float32)
        cs = np.asarray(context[c * BPC : (c + 1) * BPC], dtype=np.float32)
        in_maps.append(
            {
                "xT": np.ascontiguousarray(
                    xs.transpose(0, 2, 1).reshape(BPC * QD, NQ)
                ),
                "ctxT": np.ascontiguousarray(
                    cs.transpose(0, 2, 1).reshape(BPC * CD, NK)
                ),
                "wqT": wqT,
                "wkT": wkT,
                "wvT": wvT,
                "woT": woT,
                "bo": bo2,
            }
        )
    return in_maps


def run(inputs, trace=False):
    from concourse.bass_utils import run_bass_kernel_spmd

    if "nc" not in _CACHED:
        _CACHED["nc"] = _build_nc()
    nc = _CACHED["nc"]
    in_maps = _prep_inputs(**inputs)
    res = run_bass_kernel_spmd(nc, in_maps, list(range(NCORES)), trace=trace)
    outs = [r["out"].reshape(BPC, NQ, QD) for r in res.results]
    return np.concatenate(outs, axis=0).astype(np.float32), res


def kernel(**inputs):
    full, _ = run(inputs, trace=False)
    return full
